# revision 16
# baseline (speedup 1.0000x reference)
"""Trainium2 Bass kernel for nn_ConfidenceCalibration.

Reference computation:
    h   = x @ w1.T + b1 ; LayerNorm ; GELU
    bw  = softmax(h @ w2.T + b2, axis=-1)              # rows sum to 1
    base = sigmoid(mean(x, -1))
    scale = bin_scaling[bucket(base)] (0 out-of-range)
    out = clip(base * scale * sum(bw, -1), 0, 1)

Since softmax rows sum to exactly 1 (up to fp32 rounding ~1e-7), the MLP
branch is an algebraic no-op: out == clip(base * scale, 0, 1).  The kernel
therefore only needs a row-mean of x, a sigmoid, and a piecewise-constant
bin lookup, making it purely HBM-bound (reads x once: 128 MiB).

Sharding: data-parallel over batch; each of the 8 cores reduces a
[4096, 1024] shard.  Within a core, partition p owns rows 32p..32p+31 of
the shard so both the input DMAs (16 KiB contiguous per partition) and the
single output DMA ([128, 32] -> contiguous 4096 floats) need no transpose.

The bin lookup uses the telescoped form
    scale(v) = sum_i c_i * (v >= b_i),   c_0 = s_0, c_i = s_i - s_{i-1},
               c_NB = -s_{NB-1}
which matches searchsorted(side='right') bucketing exactly, including the
out-of-range-to-0 behavior at v < 0 and v >= 1.  The c_i come from the
runtime bin_scaling values (compilation is memoized on them).
"""

import numpy as np

B, D = 32768, 1024
N_CORES = 8
BPC = B // N_CORES  # 4096 rows per core
P = 128  # SBUF partitions
RPP = BPC // P  # 32 rows per partition
NT = 32  # input tiles per core: more/smaller DMAs shrink the serial
         # head (first DMA before the first reduce) and tail (last reduce)
R = RPP // NT  # rows-per-partition per tile
NB = 15

# Exact fp32 bits of jnp.linspace(0.0, 1.0, 16) (differs from
# np.linspace(f64).astype(f32) by 1 ulp on several entries).
_BOUND_BITS = [
    0x00000000, 0x3D888889, 0x3E088889, 0x3E4CCCCE,
    0x3E888889, 0x3EAAAAAB, 0x3ECCCCCE, 0x3EEEEEF0,
    0x3F088889, 0x3F19999A, 0x3F2AAAAB, 0x3F3BBBBC,
    0x3F4CCCCE, 0x3F5DDDDF, 0x3F6EEEF0, 0x3F800000,
]
BOUNDARIES = np.array(_BOUND_BITS, dtype=np.uint32).view(np.float32)


def build_nc(coeffs, nt=None, repeat=1, ep_splits=2):
    """Build the per-core Bass program. coeffs: 16 fp32 telescoped bin deltas.

    repeat>1 re-executes the whole body N times inside one NEFF — used only
    for wall-clock differential timing (per-iteration time = slope).
    ep_splits: process the epilogue (sigmoid/bin-scale/clip/store) in this
    many column chunks so early chunks overlap the remaining reduces.
    """
    nt = NT if nt is None else nt
    r = RPP // nt
    import concourse.bacc as bacc
    import concourse.mybir as mybir
    from concourse.tile import TileContext

    f32 = mybir.dt.float32
    # Bacc (not raw Bass): its compile() runs generate_event_semaphores,
    # which splits multi-sem sync waits into chains — hardware allows at
    # most 1 wait per instruction (2 on InstEventSemaphore).
    nc = bacc.Bacc()
    x = nc.dram_tensor("x", [BPC, D], f32, kind="ExternalInput")
    y = nc.dram_tensor("y", [BPC], f32, kind="ExternalOutput")
    xv = x.rearrange("(p c) d -> p c d", p=P)  # [128, 32, 1024]
    yv = y.rearrange("(p c) -> p c", p=P)  # [128, 32]

    with TileContext(nc) as tc:
        # Enough bufs to keep DMA streaming ahead of the DVE reduces
        # (16 x 8 KiB/partition at NT=16 -> 128 KiB/partition).
        with (
            tc.tile_pool(name="xin", bufs=min(nt, 16)) as xpool,
            tc.tile_pool(name="small", bufs=1) as spool,
        ):
          terms = [
              (float(b), float(c))
              for b, c in zip(BOUNDARIES, coeffs)
              if c != 0.0
          ]
          for _rep in range(repeat):
            acc = spool.tile([P, RPP], f32, tag="acc")
            base = spool.tile([P, RPP], f32, tag="base")
            scale = spool.tile([P, RPP], f32, tag="scale")
            tmp = spool.tile([P, RPP], f32, tag="tmp")
            out_t = spool.tile([P, RPP], f32, tag="out")

            ep_done = 0  # columns already through the epilogue
            for n in range(nt):
                xt = xpool.tile([P, r * D], f32, tag="xt")
                xt3 = xt[:].rearrange("p (r d) -> p r d", d=D)
                nc.sync.dma_start(out=xt3, in_=xv[:, n * r : (n + 1) * r, :])
                nc.vector.reduce_sum(
                    acc[:, n * r : (n + 1) * r], xt3, axis=mybir.AxisListType.X
                )

                # Run the epilogue for finished column chunks while the
                # remaining tiles are still streaming/reducing.
                cols_ready = (n + 1) * r
                chunk_end = (
                    RPP
                    if n == nt - 1
                    else (cols_ready // (RPP // ep_splits)) * (RPP // ep_splits)
                )
                if chunk_end <= ep_done:
                    continue
                cs = slice(ep_done, chunk_end)
                ep_done = chunk_end

                # base = sigmoid(acc / D); /D is an exact power-of-2 scale.
                nc.scalar.activation(
                    base[:, cs], acc[:, cs],
                    mybir.ActivationFunctionType.Sigmoid, scale=1.0 / D,
                )
                # scale = sum_i c_i * (base >= b_i)  (telescoped bin lookup)
                if not terms:
                    nc.vector.memset(scale[:, cs], 0.0)
                for k, (b, c) in enumerate(terms):
                    tgt = scale if k == 0 else tmp
                    nc.vector.tensor_scalar(
                        tgt[:, cs], base[:, cs], b, c,
                        op0=mybir.AluOpType.is_ge, op1=mybir.AluOpType.mult,
                    )
                    if k > 0:
                        nc.vector.tensor_add(scale[:, cs], scale[:, cs], tmp[:, cs])
                # out = clip(base * scale, 0, 1)
                nc.vector.tensor_mul(out_t[:, cs], base[:, cs], scale[:, cs])
                nc.vector.tensor_scalar(
                    out_t[:, cs], out_t[:, cs], 0.0, 1.0,
                    op0=mybir.AluOpType.max, op1=mybir.AluOpType.min,
                )
                # SWDGE (gpsimd) store: stays off the busy qSPDynamicHW ring.
                nc.gpsimd.dma_start(out=yv[:, cs], in_=out_t[:, cs])
    nc.compile()
    return nc


def _coeffs_from_bin_scaling(bin_scaling):
    s = np.asarray(bin_scaling, dtype=np.float32)
    c = np.zeros(NB + 1, dtype=np.float32)
    c[0] = s[0]
    c[1:NB] = s[1:] - s[:-1]
    c[NB] = -s[NB - 1]
    return c

_nc_cache = {}


def kernel(x, w1, b1, ln_g, ln_b, w2, b2, bin_scaling):
    from concourse.bass_utils import run_bass_kernel_spmd

    x = np.ascontiguousarray(np.asarray(x, dtype=np.float32))
    coeffs = _coeffs_from_bin_scaling(bin_scaling)
    key = coeffs.tobytes()
    if key not in _nc_cache:
        _nc_cache[key] = build_nc(coeffs)
    nc = _nc_cache[key]

    in_maps = [
        {"x": x[i * BPC : (i + 1) * BPC]} for i in range(N_CORES)
    ]
    res = run_bass_kernel_spmd(nc, in_maps, core_ids=list(range(N_CORES)))
    return np.concatenate([r["y"] for r in res.results])


# revision 21
# speedup vs baseline: 1.1041x; 1.1041x over previous
"""Trainium2 Bass kernel for nn_ConfidenceCalibration.

Reference computation:
    h   = x @ w1.T + b1 ; LayerNorm ; GELU
    bw  = softmax(h @ w2.T + b2, axis=-1)              # rows sum to 1
    base = sigmoid(mean(x, -1))
    scale = bin_scaling[bucket(base)] (0 out-of-range)
    out = clip(base * scale * sum(bw, -1), 0, 1)

Since softmax rows sum to exactly 1 (up to fp32 rounding ~1e-7), the MLP
branch is an algebraic no-op: out == clip(base * scale, 0, 1).  The kernel
therefore only needs a row-mean of x, a sigmoid, and a piecewise-constant
bin lookup, making it purely HBM-bound (reads x once: 128 MiB).

Sharding: data-parallel over batch; each of the 8 cores reduces a
[4096, 1024] shard.  Within a core, partition p owns rows 32p..32p+31 of
the shard so both the input DMAs (16 KiB contiguous per partition) and the
single output DMA ([128, 32] -> contiguous 4096 floats) need no transpose.

The bin lookup uses the telescoped form
    scale(v) = sum_i c_i * (v >= b_i),   c_0 = s_0, c_i = s_i - s_{i-1},
               c_NB = -s_{NB-1}
which matches searchsorted(side='right') bucketing exactly, including the
out-of-range-to-0 behavior at v < 0 and v >= 1.  The c_i come from the
runtime bin_scaling values (compilation is memoized on them).
"""

import numpy as np

B, D = 32768, 1024
N_CORES = 8
BPC = B // N_CORES  # 4096 rows per core
P = 128  # SBUF partitions
RPP = BPC // P  # 32 rows per partition
NT = 32  # input tiles per core: more/smaller DMAs shrink the serial
         # head (first DMA before the first reduce) and tail (last reduce)
R = RPP // NT  # rows-per-partition per tile
NB = 15

# Exact fp32 bits of jnp.linspace(0.0, 1.0, 16) (differs from
# np.linspace(f64).astype(f32) by 1 ulp on several entries).
_BOUND_BITS = [
    0x00000000, 0x3D888889, 0x3E088889, 0x3E4CCCCE,
    0x3E888889, 0x3EAAAAAB, 0x3ECCCCCE, 0x3EEEEEF0,
    0x3F088889, 0x3F19999A, 0x3F2AAAAB, 0x3F3BBBBC,
    0x3F4CCCCE, 0x3F5DDDDF, 0x3F6EEEF0, 0x3F800000,
]
BOUNDARIES = np.array(_BOUND_BITS, dtype=np.uint32).view(np.float32)


def build_nc(coeffs, nt=None, repeat=1, ep_splits=2, bufs=None, out_eng="sync"):
    """Build the per-core Bass program. coeffs: 16 fp32 telescoped bin deltas.

    repeat>1 re-executes the whole body N times inside one NEFF — used only
    for wall-clock differential timing (per-iteration time = slope).
    ep_splits: process the epilogue (sigmoid/bin-scale/clip/store) in this
    many column chunks so early chunks overlap the remaining reduces.
    """
    nt = NT if nt is None else nt
    r = RPP // nt
    bufs = min(nt, 16) if bufs is None else bufs
    import concourse.bacc as bacc
    import concourse.mybir as mybir
    from concourse.tile import TileContext

    f32 = mybir.dt.float32
    # Bacc (not raw Bass): its compile() runs generate_event_semaphores,
    # which splits multi-sem sync waits into chains — hardware allows at
    # most 1 wait per instruction (2 on InstEventSemaphore).
    nc = bacc.Bacc()
    x = nc.dram_tensor("x", [BPC, D], f32, kind="ExternalInput")
    y = nc.dram_tensor("y", [BPC], f32, kind="ExternalOutput")
    xv = x.rearrange("(p c) d -> p c d", p=P)  # [128, 32, 1024]
    yv = y.rearrange("(p c) -> p c", p=P)  # [128, 32]

    with TileContext(nc) as tc:
        # Enough bufs to keep DMA streaming ahead of the DVE reduces
        # (16 x 8 KiB/partition at NT=16 -> 128 KiB/partition).
        with (
            tc.tile_pool(name="xin", bufs=bufs) as xpool,
            tc.tile_pool(name="small", bufs=1) as spool,
        ):
          terms = [
              (float(b), float(c))
              for b, c in zip(BOUNDARIES, coeffs)
              if c != 0.0
          ]
          for _rep in range(repeat):
            acc = spool.tile([P, RPP], f32, tag="acc")
            base = spool.tile([P, RPP], f32, tag="base")
            scale = spool.tile([P, RPP], f32, tag="scale")
            tmp = spool.tile([P, RPP], f32, tag="tmp")
            out_t = spool.tile([P, RPP], f32, tag="out")

            ep_done = 0  # columns already through the epilogue
            for n in range(nt):
                xt = xpool.tile([P, r * D], f32, tag="xt")
                xt3 = xt[:].rearrange("p (r d) -> p r d", d=D)
                nc.sync.dma_start(out=xt3, in_=xv[:, n * r : (n + 1) * r, :])
                nc.vector.reduce_sum(
                    acc[:, n * r : (n + 1) * r], xt3, axis=mybir.AxisListType.X
                )

                # Run the epilogue for finished column chunks while the
                # remaining tiles are still streaming/reducing.
                cols_ready = (n + 1) * r
                chunk_end = (
                    RPP
                    if n == nt - 1
                    else (cols_ready // (RPP // ep_splits)) * (RPP // ep_splits)
                )
                if chunk_end <= ep_done:
                    continue
                cs = slice(ep_done, chunk_end)
                ep_done = chunk_end

                # base = sigmoid(acc / D); /D is an exact power-of-2 scale.
                nc.scalar.activation(
                    base[:, cs], acc[:, cs],
                    mybir.ActivationFunctionType.Sigmoid, scale=1.0 / D,
                )
                # scale = sum_i c_i * (base >= b_i)  (telescoped bin lookup)
                if not terms:
                    nc.vector.memset(scale[:, cs], 0.0)
                for k, (b, c) in enumerate(terms):
                    tgt = scale if k == 0 else tmp
                    nc.vector.tensor_scalar(
                        tgt[:, cs], base[:, cs], b, c,
                        op0=mybir.AluOpType.is_ge, op1=mybir.AluOpType.mult,
                    )
                    if k > 0:
                        nc.vector.tensor_add(scale[:, cs], scale[:, cs], tmp[:, cs])
                # out = clip(base * scale, 0, 1)
                nc.vector.tensor_mul(out_t[:, cs], base[:, cs], scale[:, cs])
                nc.vector.tensor_scalar(
                    out_t[:, cs], out_t[:, cs], 0.0, 1.0,
                    op0=mybir.AluOpType.max, op1=mybir.AluOpType.min,
                )
                # HWDGE (sync) store: lower latency than the SWDGE/Q7 path;
                # it queues behind the input DMAs on qSPDynamicHW but also
                # depends on the last reduce, so nothing is lost.
                store_eng = nc.gpsimd if out_eng == "gpsimd" else nc.sync
                store_eng.dma_start(out=yv[:, cs], in_=out_t[:, cs])
    nc.compile()
    return nc


def _coeffs_from_bin_scaling(bin_scaling):
    s = np.asarray(bin_scaling, dtype=np.float32)
    c = np.zeros(NB + 1, dtype=np.float32)
    c[0] = s[0]
    c[1:NB] = s[1:] - s[:-1]
    c[NB] = -s[NB - 1]
    return c

_nc_cache = {}


def kernel(x, w1, b1, ln_g, ln_b, w2, b2, bin_scaling):
    from concourse.bass_utils import run_bass_kernel_spmd

    x = np.ascontiguousarray(np.asarray(x, dtype=np.float32))
    coeffs = _coeffs_from_bin_scaling(bin_scaling)
    key = coeffs.tobytes()
    if key not in _nc_cache:
        _nc_cache[key] = build_nc(coeffs)
    nc = _nc_cache[key]

    in_maps = [
        {"x": x[i * BPC : (i + 1) * BPC]} for i in range(N_CORES)
    ]
    res = run_bass_kernel_spmd(nc, in_maps, core_ids=list(range(N_CORES)))
    return np.concatenate([r["y"] for r in res.results])


# revision 22
# speedup vs baseline: 1.1759x; 1.0650x over previous
"""Trainium2 Bass kernel for nn_ConfidenceCalibration.

Reference computation:
    h   = x @ w1.T + b1 ; LayerNorm ; GELU
    bw  = softmax(h @ w2.T + b2, axis=-1)              # rows sum to 1
    base = sigmoid(mean(x, -1))
    scale = bin_scaling[bucket(base)] (0 out-of-range)
    out = clip(base * scale * sum(bw, -1), 0, 1)

Since softmax rows sum to exactly 1 (up to fp32 rounding ~1e-7), the MLP
branch is an algebraic no-op: out == clip(base * scale, 0, 1).  The kernel
therefore only needs a row-mean of x, a sigmoid, and a piecewise-constant
bin lookup, making it purely HBM-bound (reads x once: 128 MiB).

Sharding: data-parallel over batch; each of the 8 cores reduces a
[4096, 1024] shard.  Within a core, partition p owns rows 32p..32p+31 of
the shard so both the input DMAs (16 KiB contiguous per partition) and the
single output DMA ([128, 32] -> contiguous 4096 floats) need no transpose.

The bin lookup uses the telescoped form
    scale(v) = sum_i c_i * (v >= b_i),   c_0 = s_0, c_i = s_i - s_{i-1},
               c_NB = -s_{NB-1}
which matches searchsorted(side='right') bucketing exactly, including the
out-of-range-to-0 behavior at v < 0 and v >= 1.  The c_i come from the
runtime bin_scaling values (compilation is memoized on them).
"""

import numpy as np

B, D = 32768, 1024
N_CORES = 8
BPC = B // N_CORES  # 4096 rows per core
P = 128  # SBUF partitions
RPP = BPC // P  # 32 rows per partition
NT = 32  # input tiles per core: more/smaller DMAs shrink the serial
         # head (first DMA before the first reduce) and tail (last reduce)
R = RPP // NT  # rows-per-partition per tile
NB = 15

# Exact fp32 bits of jnp.linspace(0.0, 1.0, 16) (differs from
# np.linspace(f64).astype(f32) by 1 ulp on several entries).
_BOUND_BITS = [
    0x00000000, 0x3D888889, 0x3E088889, 0x3E4CCCCE,
    0x3E888889, 0x3EAAAAAB, 0x3ECCCCCE, 0x3EEEEEF0,
    0x3F088889, 0x3F19999A, 0x3F2AAAAB, 0x3F3BBBBC,
    0x3F4CCCCE, 0x3F5DDDDF, 0x3F6EEEF0, 0x3F800000,
]
BOUNDARIES = np.array(_BOUND_BITS, dtype=np.uint32).view(np.float32)


def build_nc(coeffs, nt=None, repeat=1, ep_splits=2, bufs=None, out_eng="sync"):
    """Build the per-core Bass program. coeffs: 16 fp32 telescoped bin deltas.

    repeat>1 re-executes the whole body N times inside one NEFF — used only
    for wall-clock differential timing (per-iteration time = slope).
    ep_splits: process the epilogue (sigmoid/bin-scale/clip/store) in this
    many column chunks so early chunks overlap the remaining reduces.
    """
    nt = NT if nt is None else nt
    r = RPP // nt
    bufs = min(nt, 16) if bufs is None else bufs
    import concourse.bacc as bacc
    import concourse.mybir as mybir
    from concourse.tile import TileContext

    f32 = mybir.dt.float32
    # Bacc (not raw Bass): its compile() runs generate_event_semaphores,
    # which splits multi-sem sync waits into chains — hardware allows at
    # most 1 wait per instruction (2 on InstEventSemaphore).
    nc = bacc.Bacc()
    x = nc.dram_tensor("x", [BPC, D], f32, kind="ExternalInput")
    y = nc.dram_tensor("y", [BPC], f32, kind="ExternalOutput")
    xv = x.rearrange("(p c) d -> p c d", p=P)  # [128, 32, 1024]
    yv = y.rearrange("(p c) -> p c", p=P)  # [128, 32]

    with TileContext(nc) as tc:
        # Enough bufs to keep DMA streaming ahead of the DVE reduces
        # (16 x 8 KiB/partition at NT=16 -> 128 KiB/partition).
        with (
            tc.tile_pool(name="xin", bufs=bufs) as xpool,
            tc.tile_pool(name="small", bufs=1) as spool,
        ):
          terms = [
              (float(b), float(c))
              for b, c in zip(BOUNDARIES, coeffs)
              if c != 0.0
          ]
          for _rep in range(repeat):
            acc = spool.tile([P, RPP], f32, tag="acc")
            base = spool.tile([P, RPP], f32, tag="base")
            scale = spool.tile([P, RPP], f32, tag="scale")
            tmp = spool.tile([P, RPP], f32, tag="tmp")
            out_t = spool.tile([P, RPP], f32, tag="out")

            pa = spool.tile([P, 4], f32, tag="pa")  # head/tail partial sums

            ep_done = 0  # columns already through the epilogue
            for n in range(nt):
                split = r == 1 and n in (0, nt - 1)
                if split:
                    # Head/tail tiles stream in two 512-column halves so the
                    # first reduce starts ~0.7 us earlier and the last
                    # reduce leaves only ~0.6 us + one add past the final
                    # DMA byte, shortening the serial head/tail.
                    po = 0 if n == 0 else 2
                    for h in range(2):
                        xh = xpool.tile([P, D // 2], f32, tag="xt")
                        nc.sync.dma_start(
                            out=xh[:],
                            in_=xv[:, n, h * (D // 2) : (h + 1) * (D // 2)],
                        )
                        nc.vector.reduce_sum(
                            pa[:, po + h : po + h + 1], xh[:],
                            axis=mybir.AxisListType.X,
                        )
                    nc.vector.tensor_add(
                        acc[:, n : n + 1], pa[:, po : po + 1],
                        pa[:, po + 1 : po + 2],
                    )
                else:
                    xt = xpool.tile([P, r * D], f32, tag="xt")
                    xt3 = xt[:].rearrange("p (r d) -> p r d", d=D)
                    nc.sync.dma_start(out=xt3, in_=xv[:, n * r : (n + 1) * r, :])
                    nc.vector.reduce_sum(
                        acc[:, n * r : (n + 1) * r], xt3, axis=mybir.AxisListType.X
                    )

                # Run the epilogue for finished column chunks while the
                # remaining tiles are still streaming/reducing.
                cols_ready = (n + 1) * r
                chunk_end = (
                    RPP
                    if n == nt - 1
                    else (cols_ready // (RPP // ep_splits)) * (RPP // ep_splits)
                )
                if chunk_end <= ep_done:
                    continue
                cs = slice(ep_done, chunk_end)
                ep_done = chunk_end

                # base = sigmoid(acc / D); /D is an exact power-of-2 scale.
                nc.scalar.activation(
                    base[:, cs], acc[:, cs],
                    mybir.ActivationFunctionType.Sigmoid, scale=1.0 / D,
                )
                # scale = sum_i c_i * (base >= b_i)  (telescoped bin lookup)
                if not terms:
                    nc.vector.memset(scale[:, cs], 0.0)
                for k, (b, c) in enumerate(terms):
                    tgt = scale if k == 0 else tmp
                    nc.vector.tensor_scalar(
                        tgt[:, cs], base[:, cs], b, c,
                        op0=mybir.AluOpType.is_ge, op1=mybir.AluOpType.mult,
                    )
                    if k > 0:
                        nc.vector.tensor_add(scale[:, cs], scale[:, cs], tmp[:, cs])
                # out = clip(base * scale, 0, 1)
                nc.vector.tensor_mul(out_t[:, cs], base[:, cs], scale[:, cs])
                nc.vector.tensor_scalar(
                    out_t[:, cs], out_t[:, cs], 0.0, 1.0,
                    op0=mybir.AluOpType.max, op1=mybir.AluOpType.min,
                )
                # HWDGE (sync) store: lower latency than the SWDGE/Q7 path;
                # it queues behind the input DMAs on qSPDynamicHW but also
                # depends on the last reduce, so nothing is lost.
                store_eng = nc.gpsimd if out_eng == "gpsimd" else nc.sync
                store_eng.dma_start(out=yv[:, cs], in_=out_t[:, cs])
    nc.compile()
    return nc


def _coeffs_from_bin_scaling(bin_scaling):
    s = np.asarray(bin_scaling, dtype=np.float32)
    c = np.zeros(NB + 1, dtype=np.float32)
    c[0] = s[0]
    c[1:NB] = s[1:] - s[:-1]
    c[NB] = -s[NB - 1]
    return c

_nc_cache = {}


def kernel(x, w1, b1, ln_g, ln_b, w2, b2, bin_scaling):
    from concourse.bass_utils import run_bass_kernel_spmd

    x = np.ascontiguousarray(np.asarray(x, dtype=np.float32))
    coeffs = _coeffs_from_bin_scaling(bin_scaling)
    key = coeffs.tobytes()
    if key not in _nc_cache:
        _nc_cache[key] = build_nc(coeffs)
    nc = _nc_cache[key]

    in_maps = [
        {"x": x[i * BPC : (i + 1) * BPC]} for i in range(N_CORES)
    ]
    res = run_bass_kernel_spmd(nc, in_maps, core_ids=list(range(N_CORES)))
    return np.concatenate([r["y"] for r in res.results])
